# revision 2
# baseline (speedup 1.0000x reference)
"""Bass/Tile TRN2 kernel for nn_Attention_3264175145281.

Computes, for each batch row b:
    energy[s] = encoder_outputs[b, s, :] @ W[0, :512]   (+ const(b), dropped)
    weights   = softmax(energy)
    context   = weights @ encoder_outputs[b]

The reference adds `hidden @ W[0, 512:] + bias` to every energy[s]; that term
is constant along s, and softmax is shift-invariant, so the output does not
depend on it.  We therefore stream encoder_outputs exactly once per core.

Sharding: batch dim across 8 NeuronCores (4 rows each), W replicated.

v2 design (DMA-roofline targeted, ~420 GB/s/core observed):
  - x streamed in 1 MiB groups [128p, 4, 512] f32 (8 KiB/partition descs).
  - Each group is cast f32->bf16 immediately (mostly on the otherwise-idle
    scalar engine; one group per row on the DVE to balance engine load).
    The f32 tile is freed right after the cast -> deep DMA prefetch.
  - Energy dot products on the DVE in bf16 (2x perf mode): STT with
    accum_out, 424 ns/chunk instead of 690 ns fp32.
  - exp waves + PE context matmuls consume the bf16 copy at row end, so the
    in-order scalar/vector queues never block behind cross-engine waits.
  - Row tails (1/Z scale + output DMA) are deferred into the next row.
"""

import os
import sys

import numpy as np

for _p in ("/opt/trn_rl_repo", os.path.expanduser("~/.axon_site/_ro/trn_rl_repo")):
    if os.path.isdir(_p) and _p not in sys.path:
        sys.path.insert(0, _p)

from contextlib import ExitStack

import concourse.bacc as bacc
import concourse.bass as bass
import concourse.mybir as mybir
import concourse.tile as tile
from concourse.bass_utils import run_bass_kernel_spmd

B, S, ENC = 32, 4096, 512
NCORES = 8
B_LOC = B // NCORES          # 4 batch rows per core
P = 128                      # SBUF partitions
GRP = 4                      # chunks per DMA group (1 MiB transfers)
NGRP = S // (P * GRP)        # 8 group DMAs per batch row
NCH = NGRP * GRP             # 32 chunks of 128 positions
WAVE = 8                     # chunks per exp wave
NW = NCH // WAVE             # 4 waves per batch row
DVE_CONV_G = (3,)            # groups per row whose f32->bf16 cast runs on DVE
F32 = mybir.dt.float32
BF16 = mybir.dt.bfloat16


def build_program(n_b: int = B_LOC) -> bass.Bass:
    nc = bacc.Bacc("TRN2", target_bir_lowering=False, debug=False)

    x = nc.dram_tensor("x", [n_b, S, ENC], F32, kind="ExternalInput").ap()
    wenc = nc.dram_tensor("wenc", [1, ENC], F32, kind="ExternalInput").ap()
    out = nc.dram_tensor("out", [n_b, ENC], F32, kind="ExternalOutput").ap()

    with tile.TileContext(nc) as tc, ExitStack() as ctx:
        const_pool = ctx.enter_context(tc.tile_pool(name="const", bufs=1))
        gx_pool = ctx.enter_context(tc.tile_pool(name="gx", bufs=11))
        xb_pool = ctx.enter_context(tc.tile_pool(name="xb", bufs=11))
        scr_pool = ctx.enter_context(tc.tile_pool(name="scr", bufs=4))
        stat_pool = ctx.enter_context(tc.tile_pool(name="stat", bufs=2))
        pt_pool = ctx.enter_context(tc.tile_pool(name="pt", bufs=2))
        rs_pool = ctx.enter_context(tc.tile_pool(name="rs", bufs=2 * NW))
        tail_pool = ctx.enter_context(tc.tile_pool(name="tailp", bufs=4))
        psum_pool = ctx.enter_context(tc.tile_pool(name="psum", bufs=4, space="PSUM"))

        # w_enc replicated to all 128 partitions, then cast to bf16 once.
        wb = const_pool.tile([P, ENC], F32, tag="wb")
        nc.sync.dma_start(wb[:], wenc[:, :].broadcast_to([P, ENC]))
        wbb = const_pool.tile([P, ENC], BF16, tag="wbb")
        nc.vector.tensor_scalar_mul(wbb[:], wb[:], 1.0)

        ones = const_pool.tile([P, 1], F32, tag="ones")
        nc.gpsimd.memset(ones[:], 1.0)

        def make_tail(b, ctx_psum, z_psum):
            def tail():
                rz = tail_pool.tile([1, 1], F32, tag="rz")
                nc.vector.reciprocal(rz[:], z_psum[:])
                ot = tail_pool.tile([1, ENC], F32, tag="ot")
                # final scale on the scalar engine: out = ctx * (1/Z)
                nc.scalar.activation(
                    ot[:], ctx_psum[:], mybir.ActivationFunctionType.Copy,
                    scale=rz[:],
                )
                nc.sync.dma_start(out[b:b + 1, :], ot[:])
            return tail

        pending_tail = None
        for b in range(n_b):
            energy = stat_pool.tile([P, NCH], F32, tag="energy")
            p_t = pt_pool.tile([P, NCH], BF16, tag="p")
            ctx_psum = psum_pool.tile([1, ENC], F32, tag="ctx")
            z_psum = psum_pool.tile([1, 1], F32, tag="z")

            xbs = []
            for g in range(NGRP):
                # s = g*P*GRP + p*GRP + k: each partition reads one
                # contiguous 8 KiB run from DRAM (1 MiB per dma_start).
                gx = gx_pool.tile([P, GRP, ENC], F32, tag="gx")
                src = x[b, g * P * GRP:(g + 1) * P * GRP, :]
                nc.sync.dma_start(gx[:], src.rearrange("(p k) e -> p k e", p=P))

                # cast the whole group f32 -> bf16; frees gx for the next DMA
                xb = xb_pool.tile([P, GRP, ENC], BF16, tag="xb")
                if g in DVE_CONV_G:
                    nc.vector.tensor_scalar_mul(xb[:], gx[:], 1.0)
                else:
                    nc.scalar.activation(
                        xb[:], gx[:], mybir.ActivationFunctionType.Copy,
                    )
                xbs.append(xb)

                for k in range(GRP):
                    j = g * GRP + k
                    scr = scr_pool.tile([P, ENC], BF16, tag="scr")
                    # energy[:, j] = sum_e xb[:, e] * w_enc[e]  (one DVE pass)
                    nc.vector.scalar_tensor_tensor(
                        out=scr[:],
                        in0=xb[:, k, :],
                        scalar=1.0,
                        in1=wbb[:],
                        op0=mybir.AluOpType.mult,
                        op1=mybir.AluOpType.mult,
                        accum_out=energy[:, j:j + 1],
                    )

                # run the previous row's tiny tail once this row is flowing
                if g == 1 and pending_tail is not None:
                    pending_tail()
                    pending_tail = None

            # exp + PE consumption at row end: keeps the scalar-engine queue
            # free of cross-engine waits while the casts stream.
            for w in range(NW):
                j0 = w * WAVE
                rowsum = rs_pool.tile([P, 1], F32, tag="rowsum")
                nc.scalar.activation(
                    p_t[:, j0:j0 + WAVE], energy[:, j0:j0 + WAVE],
                    mybir.ActivationFunctionType.Exp,
                    accum_out=rowsum[:],
                )
                nc.tensor.matmul(
                    z_psum[:], rowsum[:], ones[:],
                    start=(w == 0), stop=(w == NW - 1),
                )
                for j in range(j0, j0 + WAVE):
                    nc.tensor.matmul(
                        ctx_psum[:],
                        p_t[:, j:j + 1],
                        xbs[j // GRP][:, j % GRP, :],
                        start=(j == 0),
                        stop=(j == NCH - 1),
                    )

            pending_tail = make_tail(b, ctx_psum, z_psum)

        pending_tail()

    nc.compile()
    return nc


_CACHED_NC = None


def _get_nc() -> bass.Bass:
    global _CACHED_NC
    if _CACHED_NC is None:
        _CACHED_NC = build_program()
    return _CACHED_NC


def run(inputs: dict, trace: bool = False, **kw):
    """Shard inputs, run on 8 cores, return (full_output, BassKernelResults)."""
    x_full = np.ascontiguousarray(np.asarray(inputs["encoder_outputs"], dtype=np.float32))
    w_full = np.ascontiguousarray(np.asarray(inputs["W"], dtype=np.float32))
    wenc = np.ascontiguousarray(w_full[:, :ENC])

    nc = _get_nc()
    in_maps = [
        {"x": np.ascontiguousarray(x_full[c * B_LOC:(c + 1) * B_LOC]), "wenc": wenc}
        for c in range(NCORES)
    ]
    res = run_bass_kernel_spmd(nc, in_maps, list(range(NCORES)), trace=trace, **kw)
    out = np.concatenate([res.results[c]["out"] for c in range(NCORES)], axis=0)
    return out.astype(np.float32), res


def kernel(encoder_outputs, hidden, W, b):
    out, _ = run({"encoder_outputs": encoder_outputs, "W": W})
    return out


# revision 5
# speedup vs baseline: 1.0172x; 1.0172x over previous
"""Bass/Tile TRN2 kernel for nn_Attention_3264175145281.

Computes, for each batch row b:
    energy[s] = encoder_outputs[b, s, :] @ W[0, :512]   (+ const(b), dropped)
    weights   = softmax(energy)
    context   = weights @ encoder_outputs[b]

The reference adds `hidden @ W[0, 512:] + bias` to every energy[s]; that term
is constant along s, and softmax is shift-invariant, so the output does not
depend on it.  We therefore stream encoder_outputs exactly once per core.

Sharding: batch dim across 8 NeuronCores (4 rows each), W replicated.

v3 design: DMA floor is ~80us/core (420 GB/s observed).  The energy dot
products are the bottleneck: every element needs one multiply on a
free-dim-reducing engine, and pure-DVE fp32 costs ~100us.  So the multiply
work is split: the DVE handles 3 chunks per group, GPSIMD (otherwise idle)
handles 1.  exp waves run every 2 groups; row tails are deferred into the
next row so the in-order DVE queue never stalls on cross-engine waits.
"""

import os
import sys

import numpy as np

for _p in ("/opt/trn_rl_repo", os.path.expanduser("~/.axon_site/_ro/trn_rl_repo")):
    if os.path.isdir(_p) and _p not in sys.path:
        sys.path.insert(0, _p)

from contextlib import ExitStack

import concourse.bacc as bacc
import concourse.bass as bass
import concourse.mybir as mybir
import concourse.tile as tile
from concourse.bass_utils import run_bass_kernel_spmd

B, S, ENC = 32, 4096, 512
NCORES = 8
B_LOC = B // NCORES          # 4 batch rows per core
P = 128                      # SBUF partitions
GRP = 4                      # chunks per DMA group (1 MiB transfers)
NGRP = S // (P * GRP)        # 8 group DMAs per batch row
NCH = NGRP * GRP             # 32 chunks of 128 positions
WAVE = 8                     # chunks per exp wave (2 groups)
GP_K = (0,)                  # chunk indices within a group computed on GPSIMD
F32 = mybir.dt.float32
F32R = mybir.dt.float32r     # 1 cyc/col on PE at N>=256 (vs 4 for fp32)


def build_program(n_b: int = B_LOC) -> bass.Bass:
    nc = bacc.Bacc("TRN2", target_bir_lowering=False, debug=False)

    x = nc.dram_tensor("x", [n_b, S, ENC], F32R, kind="ExternalInput").ap()
    wenc = nc.dram_tensor("wenc", [1, ENC], F32, kind="ExternalInput").ap()
    out = nc.dram_tensor("out", [n_b, ENC], F32, kind="ExternalOutput").ap()

    with tile.TileContext(nc) as tc, ExitStack() as ctx:
        const_pool = ctx.enter_context(tc.tile_pool(name="const", bufs=1))
        gx_pool = ctx.enter_context(tc.tile_pool(name="gx", bufs=14))
        scr_pool = ctx.enter_context(tc.tile_pool(name="scr", bufs=4))
        gscr_pool = ctx.enter_context(tc.tile_pool(name="gscr", bufs=4))
        stat_pool = ctx.enter_context(tc.tile_pool(name="stat", bufs=2))
        pt_pool = ctx.enter_context(tc.tile_pool(name="pt", bufs=2))
        rs_pool = ctx.enter_context(tc.tile_pool(name="rs", bufs=8))
        tail_pool = ctx.enter_context(tc.tile_pool(name="tailp", bufs=4))
        psum_pool = ctx.enter_context(tc.tile_pool(name="psum", bufs=4, space="PSUM"))

        # w_enc replicated to all 128 partitions (step-0 DMA broadcast).
        wb = const_pool.tile([P, ENC], F32, tag="wb")
        nc.sync.dma_start(wb[:], wenc[:, :].broadcast_to([P, ENC]))

        ones = const_pool.tile([P, 1], F32, tag="ones")
        nc.gpsimd.memset(ones[:], 1.0)

        def make_tail(b, ctx_psum, z_psum):
            def tail():
                rz = tail_pool.tile([1, 1], F32, tag="rz")
                nc.vector.reciprocal(rz[:], z_psum[:])
                ot = tail_pool.tile([1, ENC], F32, tag="ot")
                # final scale on the scalar engine: out = ctx * (1/Z)
                nc.scalar.activation(
                    ot[:], ctx_psum[:], mybir.ActivationFunctionType.Copy,
                    scale=rz[:],
                )
                nc.sync.dma_start(out[b:b + 1, :], ot[:])
            return tail

        def stt(gx, k, energy, j):
            # energy[:, j] = sum_e x[:, e] * w_enc[e]  (one DVE pass)
            scr = scr_pool.tile([P, ENC], F32, tag="scr")
            nc.vector.scalar_tensor_tensor(
                out=scr[:],
                in0=gx[:, k, :].bitcast(F32),
                scalar=1.0,
                in1=wb[:],
                op0=mybir.AluOpType.mult,
                op1=mybir.AluOpType.mult,
                accum_out=energy[:, j:j + 1],
            )

        def gp_stt(gx, k, energy, j):
            # same dot product, split over the idle engines: multiply on
            # GPSIMD, free-dim reduction on the scalar engine (walrus
            # rejects TensorScalarPtr on Pool, so no accum there).
            gscr = gscr_pool.tile([P, ENC], F32, tag="gscr")
            nc.gpsimd.tensor_tensor(
                gscr[:], gx[:, k, :].bitcast(F32), wb[:], mybir.AluOpType.mult,
            )
            gscr2 = gscr_pool.tile([P, ENC], F32, tag="gscr2")
            nc.scalar.activation(
                gscr2[:], gscr[:], mybir.ActivationFunctionType.Copy,
                accum_out=energy[:, j:j + 1],
            )

        pending_tail = None
        for b in range(n_b):
            groups = []
            energy = stat_pool.tile([P, NCH], F32, tag="energy")
            p_t = pt_pool.tile([P, NCH], F32R, tag="p")
            ctx_psum = psum_pool.tile([1, ENC], F32, tag="ctx")
            z_psum = psum_pool.tile([1, 1], F32, tag="z")

            for g in range(NGRP):
                # s = g*P*GRP + p*GRP + k: each partition reads one
                # contiguous 8 KiB run from DRAM (1 MiB per dma_start).
                gx = gx_pool.tile([P, GRP, ENC], F32R, tag="gx")
                src = x[b, g * P * GRP:(g + 1) * P * GRP, :]
                nc.sync.dma_start(gx[:], src.rearrange("(p k) e -> p k e", p=P))
                groups.append(gx)

                # one chunk of each group on GPSIMD+ACT, the rest on the DVE
                for k in GP_K:
                    gp_stt(gx, k, energy, g * GRP + k)
                for k in range(GRP):
                    if k in GP_K:
                        continue
                    stt(gx, k, energy, g * GRP + k)

                if g == 1 and pending_tail is not None:
                    pending_tail()
                    pending_tail = None

                # After every 2 groups: exp wave + matmul wave, so the PE
                # work overlaps the next groups' DMA/DVE instead of
                # serializing at the batch-row tail.
                if g % 2 == 1:
                    w = g // 2
                    j0 = w * WAVE
                    rowsum = rs_pool.tile([P, 1], F32, tag="rowsum")
                    nc.scalar.activation(
                        p_t[:, j0:j0 + WAVE], energy[:, j0:j0 + WAVE],
                        mybir.ActivationFunctionType.Exp,
                        accum_out=rowsum[:],
                    )
                    nc.tensor.matmul(
                        z_psum[:], rowsum[:], ones[:],
                        start=(w == 0), stop=(w == NGRP // 2 - 1),
                    )
                    for j in range(j0, j0 + WAVE):
                        nc.tensor.matmul(
                            ctx_psum[:],
                            p_t[:, j:j + 1],
                            groups[j // GRP][:, j % GRP, :],
                            start=(j == 0),
                            stop=(j == NCH - 1),
                        )

            pending_tail = make_tail(b, ctx_psum, z_psum)

        pending_tail()

    nc.compile()
    return nc


_CACHED_NC = None


def _get_nc() -> bass.Bass:
    global _CACHED_NC
    if _CACHED_NC is None:
        _CACHED_NC = build_program()
    return _CACHED_NC


def run(inputs: dict, trace: bool = False, **kw):
    """Shard inputs, run on 8 cores, return (full_output, BassKernelResults)."""
    x_full = np.ascontiguousarray(np.asarray(inputs["encoder_outputs"], dtype=np.float32))
    w_full = np.ascontiguousarray(np.asarray(inputs["W"], dtype=np.float32))
    wenc = np.ascontiguousarray(w_full[:, :ENC])

    nc = _get_nc()
    in_maps = [
        {"x": np.ascontiguousarray(x_full[c * B_LOC:(c + 1) * B_LOC]), "wenc": wenc}
        for c in range(NCORES)
    ]
    res = run_bass_kernel_spmd(nc, in_maps, list(range(NCORES)), trace=trace, **kw)
    out = np.concatenate([res.results[c]["out"] for c in range(NCORES)], axis=0)
    return out.astype(np.float32), res


def kernel(encoder_outputs, hidden, W, b):
    out, _ = run({"encoder_outputs": encoder_outputs, "W": W})
    return out


# revision 7
# speedup vs baseline: 1.0441x; 1.0265x over previous
"""Bass/Tile TRN2 kernel for nn_Attention_3264175145281.

Computes, for each batch row b:
    energy[s] = encoder_outputs[b, s, :] @ W[0, :512]   (+ const(b), dropped)
    weights   = softmax(energy)
    context   = weights @ encoder_outputs[b]

The reference adds `hidden @ W[0, 512:] + bias` to every energy[s]; that term
is constant along s, and softmax is shift-invariant, so the output does not
depend on it.  We therefore stream encoder_outputs exactly once per core.

Sharding: batch dim across 8 NeuronCores (4 rows each), W replicated.

v4 design: the DMA floor is ~80us/core (420 GB/s observed); the binding
constraint is the DVE, which must touch every element once for the energy
dot products (the PE can only contract over partitions, which it does for
the context matmuls).  Per-chunk STT+accum costs 764ns/chunk (the +151cyc
instruction init and the accumulator read dominate).  Instead, a custom DVE
op MUL_CUMSUM_ANT (registered at import time) computes the inclusive
prefix sum of x*w over a whole 4-chunk group in ONE instruction; per-chunk
energies are recovered with a tiny [P,4] strided subtract of the running
sums at chunk boundaries.  DVE cost: 2.45us per group = 613ns/chunk, no
accumulator reads.
"""

import os
import re
import sys

import numpy as np

for _p in ("/opt/trn_rl_repo", os.path.expanduser("~/.axon_site/_ro/trn_rl_repo")):
    if os.path.isdir(_p) and _p not in sys.path:
        sys.path.insert(0, _p)

from contextlib import ExitStack

import concourse.bacc as bacc
import concourse.bass as bass
import concourse.mybir as mybir
import concourse.tile as tile
from concourse.bass_utils import run_bass_kernel_spmd

# ---- custom DVE op: out[p,t] = cumsum_t(in0[p,t] * in1[p,t]) ---------------
import concourse.dve_ops as dve_ops
from concourse.dve_ops import DveOp
from concourse.dve_spec import AluOp as DveAluOp
from concourse.dve_spec import Spec as DveSpec
from concourse.dve_spec import Src0, Src1, lower as dve_lower, scan as dve_scan
from concourse.dve_table_gen import dve_ver_for
from concourse.dve_uop import DveOpSpec


def _register_mul_cumsum() -> DveOp:
    name = "MUL_CUMSUM_ANT"
    if name in dve_ops._SUB_OPCODE_FOR_NAME:
        return next(op for op in dve_ops.OPS if op.name == name)
    spec = DveSpec(
        body=dve_scan(DveAluOp.ADD, Src0 * Src1),
        reference=lambda in0, in1, s0, s1, imm2: np.cumsum(
            in0.reshape(in0.shape[0], -1).astype(np.float32)
            * in1.reshape(in0.shape[0], -1),
            axis=-1,
            dtype=np.float32,
        ).reshape(in0.shape),
    )
    row = max(dve_ops._SUB_OPCODE_FOR_NAME.values()) + 1  # 17; rows 1..31 free
    dve_ops._SUB_OPCODE_FOR_NAME[name] = row
    shas = {}
    for ver in ("v3", "v4"):
        s = DveOpSpec(name=name, opcode=row, uops=dve_lower(spec, ver=ver), rd1_en=True)
        shas[ver] = s.sha(ver)
    op = DveOp(name, spec, subdim=False, uops_sha=shas)
    dve_ops.OPS.append(op)
    dve_ops.CUSTOM_DVE_SPECS[name] = spec
    return op


MUL_CUMSUM = _register_mul_cumsum()
# ---------------------------------------------------------------------------

B, S, ENC = 32, 4096, 512
NCORES = 8
B_LOC = B // NCORES          # 4 batch rows per core
P = 128                      # SBUF partitions
GRP = 4                      # chunks per DMA group (1 MiB transfers)
NGRP = S // (P * GRP)        # 8 group DMAs per batch row
NCH = NGRP * GRP             # 32 chunks of 128 positions
GFREE = GRP * ENC            # 2048 elements per partition per group
F32 = mybir.dt.float32
F32R = mybir.dt.float32r     # 1 cyc/col on PE at N>=256 (vs 4 for fp32)


def build_program(n_b: int = B_LOC) -> bass.Bass:
    nc = bacc.Bacc("TRN2", target_bir_lowering=False, debug=False)

    x = nc.dram_tensor("x", [n_b, S, ENC], F32R, kind="ExternalInput").ap()
    wenc = nc.dram_tensor("wenc", [1, ENC], F32, kind="ExternalInput").ap()
    out = nc.dram_tensor("out", [n_b, ENC], F32, kind="ExternalOutput").ap()

    with tile.TileContext(nc) as tc, ExitStack() as ctx:
        const_pool = ctx.enter_context(tc.tile_pool(name="const", bufs=1))
        gx_pool = ctx.enter_context(tc.tile_pool(name="gx", bufs=14))
        cum_pool = ctx.enter_context(tc.tile_pool(name="cum", bufs=4))
        stat_pool = ctx.enter_context(tc.tile_pool(name="stat", bufs=2))
        pt_pool = ctx.enter_context(tc.tile_pool(name="pt", bufs=2))
        rs_pool = ctx.enter_context(tc.tile_pool(name="rs", bufs=2 * NGRP))
        tail_pool = ctx.enter_context(tc.tile_pool(name="tailp", bufs=4))
        psum_pool = ctx.enter_context(tc.tile_pool(name="psum", bufs=4, space="PSUM"))

        # w_enc replicated to all 128 partitions and GRP chunk slots, so the
        # scan's in1 stream matches a whole group elementwise.
        wb4 = const_pool.tile([P, GRP, ENC], F32, tag="wb4")
        for k in range(GRP):
            nc.sync.dma_start(wb4[:, k, :], wenc[:, :].broadcast_to([P, ENC]))

        ones = const_pool.tile([P, 1], F32, tag="ones")
        nc.gpsimd.memset(ones[:], 1.0)

        def make_tail(b, ctx_psum, z_psum):
            def tail():
                rz = tail_pool.tile([1, 1], F32, tag="rz")
                nc.vector.reciprocal(rz[:], z_psum[:])
                ot = tail_pool.tile([1, ENC], F32, tag="ot")
                # final scale on the scalar engine: out = ctx * (1/Z)
                nc.scalar.activation(
                    ot[:], ctx_psum[:], mybir.ActivationFunctionType.Copy,
                    scale=rz[:],
                )
                nc.sync.dma_start(out[b:b + 1, :], ot[:])
            return tail

        pending_tail = None
        for b in range(n_b):
            groups = []
            energy = stat_pool.tile([P, NCH], F32, tag="energy")
            p_t = pt_pool.tile([P, NCH], F32R, tag="p")
            ctx_psum = psum_pool.tile([1, ENC], F32, tag="ctx")
            z_psum = psum_pool.tile([1, 1], F32, tag="z")

            for g in range(NGRP):
                # s = g*P*GRP + p*GRP + k: each partition reads one
                # contiguous 8 KiB run from DRAM (1 MiB per dma_start).
                gx = gx_pool.tile([P, GRP, ENC], F32R, tag="gx")
                src = x[b, g * P * GRP:(g + 1) * P * GRP, :]
                nc.sync.dma_start(gx[:], src.rearrange("(p k) e -> p k e", p=P))
                groups.append(gx)

                # cum[:, 1+t] = cumsum_t(x*w); cum[:, 0] = 0 (pad)
                cum = cum_pool.tile([P, 1 + GFREE], F32, tag="cum")
                nc.gpsimd.memset(cum[:, 0:1], 0.0)
                nc.vector._custom_dve(
                    MUL_CUMSUM,
                    out=cum[:, 1:1 + GFREE],
                    in0=gx[:].bitcast(F32),
                    in1=wb4[:],
                )
                # energy[:, 4g+k] = cum[:, 512(k+1)] - cum[:, 512k]
                j0 = g * GRP
                ends = cum[:, 1:].rearrange("p (k n) -> p k n", n=ENC)[
                    :, :, ENC - 1:ENC].rearrange("p k n -> p (k n)")
                prevs = cum[:, 0:GFREE].rearrange("p (k n) -> p k n", n=ENC)[
                    :, :, 0:1].rearrange("p k n -> p (k n)")
                nc.vector.tensor_tensor(
                    energy[:, j0:j0 + GRP], ends, prevs, mybir.AluOpType.subtract,
                )

                # per-group wave: exp + Z + context matmuls
                rowsum = rs_pool.tile([P, 1], F32, tag="rowsum")
                nc.scalar.activation(
                    p_t[:, j0:j0 + GRP], energy[:, j0:j0 + GRP],
                    mybir.ActivationFunctionType.Exp,
                    accum_out=rowsum[:],
                )
                nc.tensor.matmul(
                    z_psum[:], rowsum[:], ones[:],
                    start=(g == 0), stop=(g == NGRP - 1),
                )
                for j in range(j0, j0 + GRP):
                    nc.tensor.matmul(
                        ctx_psum[:],
                        p_t[:, j:j + 1],
                        groups[j // GRP][:, j % GRP, :],
                        start=(j == 0),
                        stop=(j == NCH - 1),
                    )

                if g == 1 and pending_tail is not None:
                    pending_tail()
                    pending_tail = None

            pending_tail = make_tail(b, ctx_psum, z_psum)

        pending_tail()

    nc.compile()
    return nc


_CACHED_NC = None


def _get_nc() -> bass.Bass:
    global _CACHED_NC
    if _CACHED_NC is None:
        _CACHED_NC = build_program()
    return _CACHED_NC


def run(inputs: dict, trace: bool = False, **kw):
    """Shard inputs, run on 8 cores, return (full_output, BassKernelResults)."""
    x_full = np.ascontiguousarray(np.asarray(inputs["encoder_outputs"], dtype=np.float32))
    w_full = np.ascontiguousarray(np.asarray(inputs["W"], dtype=np.float32))
    wenc = np.ascontiguousarray(w_full[:, :ENC])

    nc = _get_nc()
    in_maps = [
        {"x": np.ascontiguousarray(x_full[c * B_LOC:(c + 1) * B_LOC]), "wenc": wenc}
        for c in range(NCORES)
    ]
    res = run_bass_kernel_spmd(nc, in_maps, list(range(NCORES)), trace=trace, **kw)
    out = np.concatenate([res.results[c]["out"] for c in range(NCORES)], axis=0)
    return out.astype(np.float32), res


def kernel(encoder_outputs, hidden, W, b):
    out, _ = run({"encoder_outputs": encoder_outputs, "W": W})
    return out
